# revision 1
# baseline (speedup 1.0000x reference)
"""FFT depthwise conv == direct 7x7 circular depthwise conv, on 8 TRN2 cores.

out[b,i,j,c] = sum_{u,v} wf[c,u,v] * x[b,(i+u-3)%H,(j+v-3)%W,c],  wf = kernel[:, ::-1, ::-1]

Sharding: data-parallel over batch (1 image per core). Host pre-pads each
image circularly to (C, 230, 230) and ships it in bf16, so every on-device
tile load is a plain contiguous-row DMA (no wrap handling on device).

Per core: partitions = 64 channels x 2 row-halves; 3 channel groups x 4
row-strips of 28 rows, each processed as two 14-row sub-strips:
  TensorE : N_PE_TAPS taps as diagonal-weight bf16 matmuls, fp32-accumulated
            in PSUM per 2-row bank tile (8 rotating single-bank tiles; a
            sub-strip's 7 banks never reuse a slot mid-sub-strip, so PE
            never stalls on same-sub-strip merges)
  VectorE : remaining taps as bf16 2-op MACs (tensor_scalar product in 4x
            mode + tensor_tensor add in 2x mode), then merges each PSUM bank
            with the accumulator into a bf16 output tile (fused downcast)
  ScalarE : copies the 6 overlapping halo rows from the previous strip tile
            (saves ~18% of input DMA) and issues half the DMAs (2nd HWDGE)
Odd-column taps are 2-byte-misaligned in bf16 and would break the DVE 2x/4x
modes, so they always go to the PE side of the split. Input/output DMAs are
row-chunked across both HWDGE queues; the next group's first tile is
prefetched one chunk per strip to keep group transitions off the critical
path. Built as bacc.Bacc (finalize() spills excess per-instruction sync
waits onto EventSemaphore instructions; engine slots are tiny).
"""

import os
import sys

for _p in ("/opt/trn_rl_repo", "/root/.axon_site/_ro/trn_rl_repo"):
    if os.path.isdir(_p) and _p not in sys.path:
        sys.path.insert(0, _p)

import numpy as np

import concourse.bacc as bacc
import concourse.bass as bass
import concourse.mybir as mybir
from concourse.bass_utils import run_bass_kernel_spmd
from concourse.tile import TileContext

F32 = mybir.dt.float32
F32R = mybir.dt.float32r
BF16 = mybir.dt.bfloat16

B, H, W, C, K = 8, 224, 224, 192, 7
NCORES = 8
PAD = K // 2          # 3
PH, PW = H + 2 * PAD, W + 2 * PAD  # 230, 230 padded image dims
HALF = H // 2         # 112 output rows per s-half
TH = 28               # output rows per strip (per half)
NSTRIP = HALF // TH   # 7
CG = 64               # channels per group
NG = C // CG          # 3
TROWS = TH + 2 * PAD  # 22 input rows per strip
TCOLS = PW            # 230 input cols per strip

# --- engine tap split (tunable) -------------------------------------------
# Odd-v taps are 2-byte-misaligned in the bf16 tile, which would knock the
# DVE out of its 2x perf mode -- so they are listed first and always land on
# the PE side of the split.
ALL_TAPS = sorted(
    ((u, v) for u in range(K) for v in range(K)),
    key=lambda t: (t[1] % 2 == 0, t[0], t[1]),
)
N_PE_TAPS = 34        # taps done on TensorE via diagonal matmuls (>= 21)
SUB = 14              # sub-strip rows (= 7 PSUM banks)
PE_TAPS = ALL_TAPS[:N_PE_TAPS]
VEC_TAPS = ALL_TAPS[N_PE_TAPS:]
USE_F32R = False
USE_BF16 = True

# DMA row-chunking: each chunk is one dma_start on its own queue/engine
IN_ROW_CHUNKS = [(0, 9), (9, 9), (18, 8), (26, 8)]     # covers TROWS=34
IN_ROW_CHUNKS_TAIL = [(6, 10), (16, 9), (25, 9)]       # rows 6..34 (halo 0..6 copied on-chip)
OUT_ROW_CHUNKS = [(0, 7), (7, 7)]                      # covers SUB=14


def _tap_idx(u, v):
    return u * K + v


def _add_dep(from_inst, to_inst):
    """Ordering-only (no-semaphore) dependency between two instructions."""
    import bass_rust as _br

    fi = getattr(from_inst, "ins", from_inst)
    ti = getattr(to_inst, "ins", to_inst)
    _br.add_dep_helper(fi, ti, sync=False, reason="seed-after-merge ordering")


def build_nc():
    # Bacc (not plain Bass): its compile() runs generate_event_semaphores,
    # which spills excess per-instruction sync waits onto EventSemaphore
    # instructions -- engine instructions only have 1 inline wait slot.
    nc = bacc.Bacc()
    xdt = BF16 if USE_BF16 else (F32R if USE_F32R else F32)
    odt = BF16 if USE_BF16 else F32
    x_d = nc.declare_dram_parameter("x", [C, PH, PW], xdt, isOutput=False)
    wvec_d = nc.declare_dram_parameter("wvec", [128, NG * K * K], F32, isOutput=False)
    wdiag_d = nc.declare_dram_parameter(
        "wdiag", [128, NG, K * K, 128], xdt, isOutput=False
    )
    out_d = nc.declare_dram_parameter("out", [C, H, W], odt, isOutput=True)

    mult = mybir.AluOpType.mult
    add = mybir.AluOpType.add
    act_copy = mybir.ActivationFunctionType.Copy

    with TileContext(nc) as tc:
        with (
            tc.tile_pool(name="consts", bufs=1) as cpool,
            tc.tile_pool(name="wdg", bufs=3) as wpool,
            tc.tile_pool(name="xin", bufs=4) as xpool,
            tc.tile_pool(name="xpre", bufs=2) as prepool,
            tc.tile_pool(name="accdp", bufs=3) as adpool,
            tc.tile_pool(name="tmpp", bufs=2) as tppool,
            tc.tile_pool(name="outp", bufs=4) as opool,
            tc.tile_pool(name="psum", bufs=8, space="PSUM") as ppool,
        ):
            wvec_sb = cpool.tile([128, NG * K * K], F32)
            nc.sync.dma_start(out=wvec_sb[:], in_=wvec_d[:])

            prev_merge = [None]  # last DVE merge instruction of previous strip

            # preload ALL groups' diagonal weights up front so group
            # transitions never wait on a 1.6 MB weight DMA stuck behind
            # the queued input DMAs
            def issue_in_dma(dst_tile, g, t, chunks=IN_ROW_CHUNKS):
                xh = x_d.tensor if hasattr(x_d, "tensor") else x_d
                base = g * CG * PH * PW + t * TH * PW
                for ci, (ra, nr) in enumerate(chunks):
                    srcap = bass.AP(
                        xh,
                        base + ra * PW,
                        [[HALF * PW, 2], [PH * PW, CG], [PW, nr], [1, TCOLS]],
                    )
                    eng = nc.sync if ci % 2 == 0 else nc.scalar
                    eng.dma_start(out=dst_tile[:, ra:ra + nr, :], in_=srcap)

            # first input tile FIRST so DVE work starts immediately; weight
            # loads follow on both queues
            xt00 = xpool.tile([128, TROWS, TCOLS], xdt, name="xt0_0", tag="xt")
            issue_in_dma(xt00, 0, 0)
            wdgs = []
            for g in range(NG):
                wdg = wpool.tile([128, K * K, 128], xdt, name=f"wdg{g}", tag="wdg")
                hkk = (K * K) // 2
                nc.sync.dma_start(out=wdg[:, 0:hkk, :], in_=wdiag_d[:, g, 0:hkk, :])
                nc.scalar.dma_start(
                    out=wdg[:, hkk:, :], in_=wdiag_d[:, g, hkk:, :]
                )
                wdgs.append(wdg)

            pre_tiles = {}
            for g in range(NG):
                wdg = wdgs[g]
                # prefetch the NEXT group's first input tile, one chunk per
                # strip of this group, so the transition tile is ready early
                # without ever bursting the DMA queues
                if g + 1 < NG:
                    pre = prepool.tile(
                        [128, TROWS, TCOLS], xdt, name=f"xpre{g + 1}", tag="xpre"
                    )
                    pre_tiles[g + 1] = pre

                for t in range(NSTRIP):
                    if g + 1 < NG:
                        # one staggered prefetch chunk for (g+1, t=0)
                        ci = t
                        ra, nr = IN_ROW_CHUNKS[ci]
                        xh = x_d.tensor if hasattr(x_d, "tensor") else x_d
                        base = (g + 1) * CG * PH * PW
                        srcap = bass.AP(
                            xh,
                            base + ra * PW,
                            [[HALF * PW, 2], [PH * PW, CG], [PW, nr], [1, TCOLS]],
                        )
                        eng = nc.sync if ci % 2 == 0 else nc.scalar
                        eng.dma_start(
                            out=pre_tiles[g + 1][:, ra:ra + nr, :], in_=srcap
                        )
                    if g == 0 and t == 0:
                        xt = xt00
                    elif t == 0 and g in pre_tiles:
                        xt = pre_tiles.pop(g)
                    else:
                        xt = xpool.tile(
                            [128, TROWS, TCOLS], xdt, name=f"xt{g}_{t}", tag="xt"
                        )
                        # rows 0..5 overlap the previous strip's tail: copy
                        # them on-chip (idle ScalarE) instead of re-DMAing
                        issue_in_dma(xt, g, t, chunks=IN_ROW_CHUNKS_TAIL)
                        nc.scalar.copy(
                            out=xt[:, 0:2 * PAD, :],
                            in_=prev_xt[:, TH:TH + 2 * PAD, :],
                        )
                    prev_xt = xt

                    # ---- two 14-row sub-strips per DMA strip: each uses
                    # exactly 7 PSUM banks (the full rotation), so PE never
                    # stalls waiting for same-strip merges
                    for sub in range(TH // SUB):
                        sb = sub * SUB
                        acc = adpool.tile(
                            [128, SUB, W], BF16, name=f"acc{g}_{t}_{sub}", tag="acc"
                        )
                        outt = opool.tile(
                            [128, SUB, W], odt, name=f"outt{g}_{t}_{sub}", tag="outt"
                        )
                        tmps = [
                            tppool.tile(
                                [128, SUB, W], BF16,
                                name=f"tmp{g}_{t}_{sub}_{j}", tag=f"tmp{j}",
                            )
                            for j in range(2)
                        ]

                        # ---- vector taps on DVE: all-bf16 2-op MACs.
                        # tensor_scalar products run in 4x mode, tensor_tensor
                        # adds in 2x mode -- beats the 1x-capped fused STT.
                        u0, v0 = VEC_TAPS[0]
                        ti0 = g * K * K + _tap_idx(u0, v0)
                        wv0 = wvec_sb[:, ti0:ti0 + 1]
                        seed = nc.vector.tensor_scalar(
                            acc[:],
                            xt[:, u0 + sb:u0 + sb + SUB, v0:v0 + W],
                            wv0,
                            None,
                            mult,
                        )
                        if prev_merge[0] is not None:
                            _add_dep(seed, prev_merge[0])
                        for j, (u, v) in enumerate(VEC_TAPS[1:]):
                            ti = g * K * K + _tap_idx(u, v)
                            wv = wvec_sb[:, ti:ti + 1]
                            tmp = tmps[j % 2]
                            nc.vector.tensor_scalar(
                                tmp[:],
                                xt[:, u + sb:u + sb + SUB, v:v + W],
                                wv,
                                None,
                                mult,
                            )
                            nc.vector.tensor_tensor(acc[:], acc[:], tmp[:], add)

                        # ---- TensorE taps: SUB/2 bank-tiles of 2 rows ----
                        n_pe = len(PE_TAPS)
                        for b8 in range(SUB // 2):
                            ps = ppool.tile(
                                [128, 512], F32, name=f"ps{g}_{t}_{sub}_{b8}", tag="ps"
                            )
                            row0 = 2 * b8
                            for ti, (u, v) in enumerate(PE_TAPS):
                                rhs = xt[:, u + sb + row0:u + sb + row0 + 2, v:v + W]
                                nc.tensor.matmul(
                                    ps[:, 0:2 * W],
                                    wdg[:, _tap_idx(u, v), :],
                                    rhs,
                                    start=(ti == 0),
                                    stop=(ti == n_pe - 1),
                                )
                            # merge psum + acc -> bf16 output tile (DVE)
                            ps3 = ps[:, 0:2 * W].rearrange("p (r w) -> p r w", r=2)
                            mg = nc.vector.scalar_tensor_tensor(
                                outt[:, row0:row0 + 2, :],
                                ps3,
                                1.0,
                                acc[:, row0:row0 + 2, :],
                                mult,
                                add,
                            )
                            if b8 == 0:
                                # the ordering hint for the next seed points at
                                # the FIRST merge: enough to cover transitive
                                # PE ticks, without serializing the next
                                # sub-strip behind PE's last bank
                                prev_merge[0] = mg

                        # ---- output DMA per sub-strip, row-chunked ----
                        oh = out_d.tensor if hasattr(out_d, "tensor") else out_d
                        obase = g * CG * H * W + (t * TH + sb) * W
                        for ci, (ra, nr) in enumerate(OUT_ROW_CHUNKS):
                            dst = bass.AP(
                                oh,
                                obase + ra * W,
                                [[HALF * W, 2], [H * W, CG], [W, nr], [1, W]],
                            )
                            eng = nc.scalar if ci % 2 == 0 else nc.sync
                            eng.dma_start(out=dst, in_=outt[:, ra:ra + nr, :])
    return nc


def _host_weights(kernel):
    """kernel: (C, K, K) -> (wvec [128, NG*49], wdiag [128, NG, 49, 128])."""
    wf = kernel[:, ::-1, ::-1].astype(np.float32)  # flipped: cross-correlation form
    cl = np.arange(128) % CG  # channel-local index per partition
    wvec = np.empty((128, NG * K * K), dtype=np.float32)
    wdiag = np.zeros((128, NG, K * K, 128), dtype=np.float32)
    eye = np.arange(128)
    for g in range(NG):
        wg = wf[g * CG:(g + 1) * CG].reshape(CG, K * K)  # (64, 49)
        wvec[:, g * K * K:(g + 1) * K * K] = wg[cl]
        wdiag[eye, g, :, eye] = wg[cl]
    return wvec, wdiag


_NC_CACHE = {}


def _get_nc():
    if "nc" not in _NC_CACHE:
        nc = build_nc()
        # Bacc passes (register alloc, EventSemaphore wait-splitting, ...)
        # run in finalize(); the pjrt path serializes the module as-is, so
        # finalize here before handing it off.
        nc.finalize()
        _NC_CACHE["nc"] = nc
    return _NC_CACHE["nc"]


def run(x, kernel, trace=False, **kw):
    assert x.shape == (B, H, W, C) and kernel.shape == (C, K, K)
    nc = _get_nc()
    xT = np.ascontiguousarray(x.transpose(0, 3, 1, 2)).astype(np.float32)  # (B,C,H,W)
    xTp = np.pad(xT, ((0, 0), (0, 0), (PAD, PAD), (PAD, PAD)), mode="wrap")
    xTp = np.ascontiguousarray(xTp)
    wvec, wdiag = _host_weights(np.asarray(kernel))
    if USE_BF16:
        import ml_dtypes

        xTp = xTp.astype(ml_dtypes.bfloat16)
        wdiag = wdiag.astype(ml_dtypes.bfloat16)
    in_maps = [{"x": xTp[b], "wvec": wvec, "wdiag": wdiag} for b in range(NCORES)]
    res = run_bass_kernel_spmd(nc, in_maps, list(range(NCORES)), trace=trace, **kw)
    out = np.stack(
        [np.asarray(res.results[b]["out"]).astype(np.float32) for b in range(NCORES)]
    )
    out = np.ascontiguousarray(out.transpose(0, 2, 3, 1)).astype(np.float32)
    return out, res


def kernel(x, kernel):
    out, _ = run(np.asarray(x), np.asarray(kernel))
    return out



# revision 3
# speedup vs baseline: 4.3096x; 4.3096x over previous
"""FFT depthwise conv == direct 7x7 circular depthwise conv, on 8 TRN2 cores.

out[b,i,j,c] = sum_{u,v} wf[c,u,v] * x[b,(i+u-3)%H,(j+v-3)%W,c],  wf = kernel[:, ::-1, ::-1]

v2: all 49 taps on TensorE via banded-Toeplitz matmuls.

Sharding: data-parallel over batch (1 image per core). Per channel-pair
(2 channels x 64-row W-windows on the 128 partitions), the v-convolution
is one matmul with a block-diagonal banded-Toeplitz stationary matrix
T[p, m] = wf[c, u, p-m]; the 7 u-taps accumulate in PSUM fp32. Input is
host-side transposed per channel (partitions = padded W, free = padded H),
so tap u is just a free-dim offset into the same SBUF tile.

Per (pair g, window t): 7 matmuls N=224 -> psum[128, 224]; valid output
rows m in [m0_t, m0_t+nj_t) map to out columns j = W0S[t] + m for ch 2g
(partitions m) and 2g+1 (partitions 64+m). DVE/ScalarE alternate the
psum -> bf16 staging copies; host undoes all layout shuffling.

DMA: host pre-tiles x/w/out into [96, 128, free] tensors so each 8-pair
block moves as ONE dma_start of 1024 contiguous ~1.8KB descriptors --
that granularity fans out across all 16 DMA engines (~370 GB/s measured
vs ~50 GB/s for 128-descriptor batches). x+w blocks on the sync queue,
out blocks on gpsimd, leaving ScalarE/DVE free for the psum copies.
"""

import os
import sys

for _p in ("/opt/trn_rl_repo", "/root/.axon_site/_ro/trn_rl_repo"):
    if os.path.isdir(_p) and _p not in sys.path:
        sys.path.insert(0, _p)

import numpy as np

import concourse.bacc as bacc
import concourse.bass as bass
import concourse.mybir as mybir
from concourse.bass_utils import run_bass_kernel_spmd
from concourse.tile import TileContext

F32 = mybir.dt.float32
BF16 = mybir.dt.bfloat16

B, H, W, C, K = 8, 224, 224, 192, 7
NCORES = 8
PAD = K // 2                  # 3
HP = H + 2 * PAD              # 230 padded rows (free dim)
WP = W + 2 * PAD              # 230 padded cols (partition windows)
NPAIR = C // 2                # 96 channel pairs
W0S = [0, 58, 116, 166]       # window starts (input w-padded coords)
M0S = [0, 0, 0, 8]            # first valid m per window
NJS = [58, 58, 58, 50]        # valid rows per window
NT = len(W0S)                 # 4 windows
XF = NT * HP                  # 920 x free elems per (pair, partition)
WF = K * 128                  # 896 w free elems
OF = NT * H                   # 896 out free elems
BLK = 8                       # pairs per DMA block
NBLK = NPAIR // BLK           # 12


def build_nc():
    nc = bacc.Bacc()
    x_d = nc.declare_dram_parameter("x", [NPAIR, 128, XF], BF16, isOutput=False)
    w_d = nc.declare_dram_parameter("w", [NPAIR, 128, WF], BF16, isOutput=False)
    o_d = nc.declare_dram_parameter("out", [NPAIR, 128, OF], BF16, isOutput=True)
    xh = x_d.tensor if hasattr(x_d, "tensor") else x_d
    wh = w_d.tensor if hasattr(w_d, "tensor") else w_d
    oh = o_d.tensor if hasattr(o_d, "tensor") else o_d

    with TileContext(nc) as tc:
        with (
            tc.tile_pool(name="xin", bufs=2) as xpool,
            tc.tile_pool(name="wts", bufs=2) as wpool,
            tc.tile_pool(name="outp", bufs=2) as opool,
            tc.tile_pool(name="psum", bufs=8, space="PSUM") as ppool,
        ):
            for blk in range(NBLK):
                g0 = blk * BLK
                xt = xpool.tile([128, BLK, XF], BF16, name=f"x{blk}", tag="x")
                nc.sync.dma_start(
                    out=xt[:, :, :],
                    in_=bass.AP(
                        xh, g0 * 128 * XF, [[XF, 128], [128 * XF, BLK], [1, XF]]
                    ),
                )
                wt = wpool.tile([128, BLK, WF], BF16, name=f"w{blk}", tag="w")
                nc.sync.dma_start(
                    out=wt[:, :, :],
                    in_=bass.AP(
                        wh, g0 * 128 * WF, [[WF, 128], [128 * WF, BLK], [1, WF]]
                    ),
                )
                ot = opool.tile([128, BLK, OF], BF16, name=f"o{blk}", tag="o")
                for pl in range(BLK):
                    for t in range(NT):
                        ps = ppool.tile(
                            [128, 512], F32, name=f"ps{blk}_{pl}_{t}", tag="ps"
                        )
                        for u in range(K):
                            nc.tensor.matmul(
                                ps[:, 0:H],
                                wt[:, pl, u * 128:(u + 1) * 128],
                                xt[:, pl, t * HP + u:t * HP + u + H],
                                start=(u == 0),
                                stop=(u == K - 1),
                            )
                        dst = ot[:, pl, t * H:(t + 1) * H]
                        if t % 2 == 0:
                            nc.vector.tensor_scalar_mul(dst, ps[:, 0:H], 1.0)
                        else:
                            nc.scalar.copy(out=dst, in_=ps[:, 0:H])
                nc.gpsimd.dma_start(
                    out=bass.AP(
                        oh, g0 * 128 * OF, [[OF, 128], [128 * OF, BLK], [1, OF]]
                    ),
                    in_=ot[:, :, :],
                )
    return nc


def _host_x(x):
    """x: (B, H, W, C) f32 -> (B, NPAIR, 128, XF) bf16 pre-tiled."""
    import ml_dtypes

    # xT[b, c, wp, hp] = x[b, (hp-3)%H, (wp-3)%W, c]
    xT = np.ascontiguousarray(x.transpose(0, 3, 2, 1))          # (B, C, W, H)
    xT = np.pad(xT, ((0, 0), (0, 0), (PAD, PAD), (PAD, PAD)), mode="wrap")
    xT = xT.astype(ml_dtypes.bfloat16)                          # (B, C, WP, HP)
    xdev = np.empty((B, NPAIR, 128, NT, HP), dtype=ml_dtypes.bfloat16)
    for t, w0 in enumerate(W0S):
        win = xT[:, :, w0:w0 + 64, :]                           # (B, C, 64, HP)
        xdev[:, :, 0:64, t, :] = win[:, 0::2]
        xdev[:, :, 64:128, t, :] = win[:, 1::2]
    return xdev.reshape(B, NPAIR, 128, XF)


def _host_w(kernel):
    """kernel: (C, K, K) -> (NPAIR, 128, WF) bf16 block-diag Toeplitz."""
    import ml_dtypes

    wf = kernel[:, ::-1, ::-1].astype(np.float32)               # flipped taps
    wdev = np.zeros((NPAIR, 128, K, 128), dtype=np.float32)
    m = np.arange(64)
    for v in range(K):
        pm = m + v                                              # p = m + v
        ok = pm < 64
        # block 0: channel 2g, block 1: channel 2g+1
        # advanced indices (pm, m) land in front: result (ndiag, NPAIR, K)
        wdev[:, pm[ok], :, m[ok]] = wf[0::2, :, v][None, :, :]
        wdev[:, 64 + pm[ok], :, 64 + m[ok]] = wf[1::2, :, v][None, :, :]
    return np.ascontiguousarray(wdev.reshape(NPAIR, 128, WF)).astype(
        ml_dtypes.bfloat16
    )


def _host_unshuffle(odev):
    """odev: (B, NPAIR, 128, OF) -> (B, H, W, C) f32."""
    o = np.asarray(odev, dtype=np.float32).reshape(B, NPAIR, 128, NT, H)
    out = np.empty((B, H, W, C), dtype=np.float32)
    for t, w0 in enumerate(W0S):
        m0, nj = M0S[t], NJS[t]
        j0 = w0 + m0
        # o[b, g, m, t, i] -> out[b, i, j0+mm, 2g]; partitions 64+m -> 2g+1
        out[:, :, j0:j0 + nj, 0::2] = o[:, :, m0:m0 + nj, t, :].transpose(0, 3, 2, 1)
        out[:, :, j0:j0 + nj, 1::2] = o[:, :, 64 + m0:64 + m0 + nj, t, :].transpose(
            0, 3, 2, 1
        )
    return out


_NC_CACHE = {}


def _get_nc():
    if "nc" not in _NC_CACHE:
        nc = build_nc()
        nc.finalize()
        _NC_CACHE["nc"] = nc
    return _NC_CACHE["nc"]


def run(x, kernel, trace=False, **kw):
    assert x.shape == (B, H, W, C) and kernel.shape == (C, K, K)
    nc = _get_nc()
    xdev = _host_x(np.asarray(x, dtype=np.float32))
    wdev = _host_w(np.asarray(kernel))
    in_maps = [{"x": xdev[b], "w": wdev} for b in range(NCORES)]
    res = run_bass_kernel_spmd(nc, in_maps, list(range(NCORES)), trace=trace, **kw)
    odev = np.stack([np.asarray(res.results[b]["out"]) for b in range(NCORES)])
    return _host_unshuffle(odev), res


def kernel(x, kernel):
    out, _ = run(np.asarray(x), np.asarray(kernel))
    return out


# revision 4
# speedup vs baseline: 4.4903x; 1.0419x over previous
"""FFT depthwise conv == direct 7x7 circular depthwise conv, on 8 TRN2 cores.

out[b,i,j,c] = sum_{u,v} wf[c,u,v] * x[b,(i+u-3)%H,(j+v-3)%W,c],  wf = kernel[:, ::-1, ::-1]

v2: all 49 taps on TensorE via banded-Toeplitz matmuls.

Sharding: data-parallel over batch (1 image per core). Per channel-pair
(2 channels x 64-row W-windows on the 128 partitions), the v-convolution
is one matmul with a block-diagonal banded-Toeplitz stationary matrix
T[p, m] = wf[c, u, p-m]; the 7 u-taps accumulate in PSUM fp32. Input is
host-side transposed per channel (partitions = padded W, free = padded H),
so tap u is just a free-dim offset into the same SBUF tile.

Per (pair g, window t): 7 matmuls N=224 -> psum[128, 224]; valid output
rows m in [m0_t, m0_t+nj_t) map to out columns j = W0S[t] + m for ch 2g
(partitions m) and 2g+1 (partitions 64+m). DVE/ScalarE alternate the
psum -> bf16 staging copies; host undoes all layout shuffling.

DMA: host pre-tiles x/w/out into [96, 128, free] tensors so each 8-pair
block moves as ONE dma_start of 1024 contiguous ~1.8KB descriptors --
that granularity fans out across all 16 DMA engines (~370 GB/s measured
vs ~50 GB/s for 128-descriptor batches). x+w blocks on the sync queue,
out blocks on gpsimd, leaving ScalarE/DVE free for the psum copies.
"""

import os
import sys

for _p in ("/opt/trn_rl_repo", "/root/.axon_site/_ro/trn_rl_repo"):
    if os.path.isdir(_p) and _p not in sys.path:
        sys.path.insert(0, _p)

import numpy as np

import concourse.bacc as bacc
import concourse.bass as bass
import concourse.mybir as mybir
from concourse.bass_utils import run_bass_kernel_spmd
from concourse.tile import TileContext

F32 = mybir.dt.float32
BF16 = mybir.dt.bfloat16

B, H, W, C, K = 8, 224, 224, 192, 7
NCORES = 8
PAD = K // 2                  # 3
HP = H + 2 * PAD              # 230 padded rows (free dim)
WP = W + 2 * PAD              # 230 padded cols (partition windows)
NPAIR = C // 2                # 96 channel pairs
W0S = [0, 58, 116, 166]       # window starts (input w-padded coords)
M0S = [0, 0, 0, 8]            # first valid m per window
NJS = [58, 58, 58, 50]        # valid rows per window
NT = len(W0S)                 # 4 windows
XF = NT * HP                  # 920 x free elems per (pair, partition)
WF = K * 128                  # 896 w free elems
OF = NT * H                   # 896 out free elems
BLK = 8                       # max pairs per DMA block
# small edge blocks (with fine-grained DMA descriptors) shrink the startup
# wait before the first matmul and the drain after the last one
PB = [2, 6] + [8] * 10 + [6, 2]
assert sum(PB) == NPAIR


def build_nc():
    nc = bacc.Bacc()
    x_d = nc.declare_dram_parameter("x", [NPAIR, 128, XF], BF16, isOutput=False)
    w_d = nc.declare_dram_parameter("w", [NPAIR, 128, WF], BF16, isOutput=False)
    o_d = nc.declare_dram_parameter("out", [NPAIR, 128, OF], BF16, isOutput=True)
    xh = x_d.tensor if hasattr(x_d, "tensor") else x_d
    wh = w_d.tensor if hasattr(w_d, "tensor") else w_d
    oh = o_d.tensor if hasattr(o_d, "tensor") else o_d

    def xap(g0, n, fine):
        dims = (
            [[XF, 128], [128 * XF, n], [HP, NT], [1, HP]]
            if fine
            else [[XF, 128], [128 * XF, n], [1, XF]]
        )
        return bass.AP(xh, g0 * 128 * XF, dims)

    def wap(g0, n, fine):
        dims = (
            [[WF, 128], [128 * WF, n], [128, K], [1, 128]]
            if fine
            else [[WF, 128], [128 * WF, n], [1, WF]]
        )
        return bass.AP(wh, g0 * 128 * WF, dims)

    def oap(g0, n, fine):
        dims = (
            [[OF, 128], [128 * OF, n], [H, NT], [1, H]]
            if fine
            else [[OF, 128], [128 * OF, n], [1, OF]]
        )
        return bass.AP(oh, g0 * 128 * OF, dims)

    with TileContext(nc) as tc:
        with (
            tc.tile_pool(name="xin", bufs=2) as xpool,
            tc.tile_pool(name="wts", bufs=2) as wpool,
            tc.tile_pool(name="outp", bufs=2) as opool,
            tc.tile_pool(name="psum", bufs=8, space="PSUM") as ppool,
        ):
            g0 = 0
            for blk, n in enumerate(PB):
                fine = n <= 2
                xt = xpool.tile([128, BLK, XF], BF16, name=f"x{blk}", tag="x")
                nc.sync.dma_start(
                    out=(
                        xt[:, 0:n, :].rearrange("p b (t h) -> p b t h", t=NT)
                        if fine
                        else xt[:, 0:n, :]
                    ),
                    in_=xap(g0, n, fine),
                )
                wt = wpool.tile([128, BLK, WF], BF16, name=f"w{blk}", tag="w")
                nc.gpsimd.dma_start(
                    out=(
                        wt[:, 0:n, :].rearrange("p b (u m) -> p b u m", u=K)
                        if fine
                        else wt[:, 0:n, :]
                    ),
                    in_=wap(g0, n, fine),
                )
                ot = opool.tile([128, BLK, OF], BF16, name=f"o{blk}", tag="o")
                for pl in range(n):
                    for t in range(NT):
                        ps = ppool.tile(
                            [128, 512], F32, name=f"ps{blk}_{pl}_{t}", tag="ps"
                        )
                        for u in range(K):
                            nc.tensor.matmul(
                                ps[:, 0:H],
                                wt[:, pl, u * 128:(u + 1) * 128],
                                xt[:, pl, t * HP + u:t * HP + u + H],
                                start=(u == 0),
                                stop=(u == K - 1),
                            )
                        nc.vector.tensor_scalar_mul(
                            ot[:, pl, t * H:(t + 1) * H], ps[:, 0:H], 1.0
                        )
                nc.scalar.dma_start(
                    out=oap(g0, n, fine),
                    in_=(
                        ot[:, 0:n, :].rearrange("p b (t h) -> p b t h", t=NT)
                        if fine
                        else ot[:, 0:n, :]
                    ),
                )
                g0 += n
    return nc


def _host_x(x):
    """x: (B, H, W, C) f32 -> (B, NPAIR, 128, XF) bf16 pre-tiled."""
    import ml_dtypes

    # xT[b, c, wp, hp] = x[b, (hp-3)%H, (wp-3)%W, c]
    xT = np.ascontiguousarray(x.transpose(0, 3, 2, 1))          # (B, C, W, H)
    xT = np.pad(xT, ((0, 0), (0, 0), (PAD, PAD), (PAD, PAD)), mode="wrap")
    xT = xT.astype(ml_dtypes.bfloat16)                          # (B, C, WP, HP)
    xdev = np.empty((B, NPAIR, 128, NT, HP), dtype=ml_dtypes.bfloat16)
    for t, w0 in enumerate(W0S):
        win = xT[:, :, w0:w0 + 64, :]                           # (B, C, 64, HP)
        xdev[:, :, 0:64, t, :] = win[:, 0::2]
        xdev[:, :, 64:128, t, :] = win[:, 1::2]
    return xdev.reshape(B, NPAIR, 128, XF)


def _host_w(kernel):
    """kernel: (C, K, K) -> (NPAIR, 128, WF) bf16 block-diag Toeplitz."""
    import ml_dtypes

    wf = kernel[:, ::-1, ::-1].astype(np.float32)               # flipped taps
    wdev = np.zeros((NPAIR, 128, K, 128), dtype=np.float32)
    m = np.arange(64)
    for v in range(K):
        pm = m + v                                              # p = m + v
        ok = pm < 64
        # block 0: channel 2g, block 1: channel 2g+1
        # advanced indices (pm, m) land in front: result (ndiag, NPAIR, K)
        wdev[:, pm[ok], :, m[ok]] = wf[0::2, :, v][None, :, :]
        wdev[:, 64 + pm[ok], :, 64 + m[ok]] = wf[1::2, :, v][None, :, :]
    return np.ascontiguousarray(wdev.reshape(NPAIR, 128, WF)).astype(
        ml_dtypes.bfloat16
    )


def _host_unshuffle(odev):
    """odev: (B, NPAIR, 128, OF) -> (B, H, W, C) f32."""
    o = np.asarray(odev, dtype=np.float32).reshape(B, NPAIR, 128, NT, H)
    out = np.empty((B, H, W, C), dtype=np.float32)
    for t, w0 in enumerate(W0S):
        m0, nj = M0S[t], NJS[t]
        j0 = w0 + m0
        # o[b, g, m, t, i] -> out[b, i, j0+mm, 2g]; partitions 64+m -> 2g+1
        out[:, :, j0:j0 + nj, 0::2] = o[:, :, m0:m0 + nj, t, :].transpose(0, 3, 2, 1)
        out[:, :, j0:j0 + nj, 1::2] = o[:, :, 64 + m0:64 + m0 + nj, t, :].transpose(
            0, 3, 2, 1
        )
    return out


_NC_CACHE = {}


def _get_nc():
    if "nc" not in _NC_CACHE:
        nc = build_nc()
        nc.finalize()
        _NC_CACHE["nc"] = nc
    return _NC_CACHE["nc"]


def run(x, kernel, trace=False, **kw):
    assert x.shape == (B, H, W, C) and kernel.shape == (C, K, K)
    nc = _get_nc()
    xdev = _host_x(np.asarray(x, dtype=np.float32))
    wdev = _host_w(np.asarray(kernel))
    in_maps = [{"x": xdev[b], "w": wdev} for b in range(NCORES)]
    res = run_bass_kernel_spmd(nc, in_maps, list(range(NCORES)), trace=trace, **kw)
    odev = np.stack([np.asarray(res.results[b]["out"]) for b in range(NCORES)])
    return _host_unshuffle(odev), res


def kernel(x, kernel):
    out, _ = run(np.asarray(x), np.asarray(kernel))
    return out
